# revision 41
# baseline (speedup 1.0000x reference)
"""LSTM autoencoder (4-layer + TimeDistributed Dense) on 8 TRN2 NeuronCores.

Sharding: data-parallel over batch (B=256 -> 32 samples/core), weights
replicated. Per-core layout keeps everything "transposed": states are
[H partitions, batch free], so the recurrent matmul is
  z^T[gate] = W[:, gate]^T @ h^T   (weights stationary, state moving, N=32)
and the gate nonlinearities/cell updates run on [H, 32] tiles.

The kernel is latency/DVE-bound by the sequential recurrence (2048
chained steps of matmul -> sigmoid -> cell update). Structure:

1. Phase interleaving (wavefront): encoder layer B trails layer A by
   one 8-step block inside one loop; decoder layer D trails C, and the
   TimeDistributed Dense (E) rides along two blocks behind D. Trailing
   work executes inside the leading chain's dependency-latency gaps.
2. Short per-step chain: one sigmoid covers the (i,f,o) PSUM planes;
   relu(g)*i is a single DVE scalar_tensor_tensor reading the g plane
   straight from PSUM ((g max 0) mult i); f*c runs on the otherwise
   idle GPSIMD engine (SBUF operands only - GPSIMD cannot touch PSUM,
   walrus rejects it); c+u and h=o*c stay on DVE.
3. Dtypes: recurrent weights and h sequences are bfloat16 (recurrent
   matmuls 13ns, input-side N=256 matmuls 107ns); the x path (wk1) and
   dense bias stay float32r (1 cycle/row at N>=256, near-fp32 HW
   precision); cell state c and all pointwise math stay fp32. Measured
   HW rel err 5.2e-3 vs the 2e-2 gate.

Per 8-step block the input-side (Wk) matmuls are batched as one N=256
matmul per gate plane accumulated in PSUM; per-step recurrent matmuls
accumulate N=32 slices on top. PSUM "start" zeroes the whole 2KB
zero-region (bank), and with BLKC=256 each plane is half a bank: only
the first plane of a bank may assert start; its sibling accumulates
onto the freshly zeroed bytes. Keeping the trailing chains' input
matmuls at full-block granularity matters: finer granularity lets the
greedy scheduler couple the trailing chain 1 step behind the leader,
which stretches the leader's chain (measured 830ns -> 990ns rounds).

Biases ride in a ones-row augmentation of the moving operand on
whichever side has K < 128. Gate plane order is (g, i, f, o) so one
sigmoid covers planes 1:4. relu(c) == c identically because c >= 0 by
induction (g >= 0 post-relu, i,f in (0,1), c0 = 0), so h = o * c.

All input DMAs are issued up-front on one queue (x block-aligned with
a single-block first chunk so compute starts immediately); per-block
DMAs would rotate across all 8 DMA-queue semaphores and push
per-matmul sync-wait counts past the ISA limit (PE matmuls carry one
wait; _split_excess_waits spills the rest onto NoOps).
"""

import numpy as np

B, T, F, H1, H2 = 256, 512, 64, 128, 64
NCORES = 8
BC = B // NCORES          # 32 samples per core
NT = T * BC               # 16384 columns in time-major (t, b) layout
SB = 8                    # recurrence steps per PSUM block
NBLK = T // SB            # 64 blocks
BLKC = SB * BC            # 256 columns per block
PERM = [2, 0, 1, 3]       # keras (i,f,g,o) -> planes (g,i,f,o)
XCHUNKS = 4               # x input DMA split

# f32r pack (x-side weights + dense bias): (name, rows, cols)
WSEGS = [("wk1", 65, 512), ("bout", 64, 1)]
# bf16 pack (recurrent weights + everything consuming bf16 h-sequences)
BSEGS = [("wr1", 128, 512), ("wk2", 128, 256), ("wr2", 65, 256),
         ("wd1k", 65, 256), ("wd1r", 64, 256), ("wd2k", 65, 512),
         ("wd2r", 128, 512), ("wout", 128, 64)]
WOFF = {}
_o = 0
for _n, _p, _c in WSEGS:
    WOFF[_n] = _o
    _o += _c
WCOLS = _o
BOFF = {}
_o = 0
for _n, _p, _c in BSEGS:
    BOFF[_n] = _o
    _o += _c
BCOLS = _o

_CACHE = {}


def _build():
    import concourse.bass as bass
    import concourse.mybir as mybir
    import concourse.tile as tile

    f32 = mybir.dt.float32
    f32r = mybir.dt.float32r
    bf16 = mybir.dt.bfloat16
    AF = mybir.ActivationFunctionType
    ALU = mybir.AluOpType

    nc = bass.Bass()

    xa = nc.dram_tensor("xa", [F + 1, NT], f32r, kind="ExternalInput")
    wp_d = nc.dram_tensor("wpack", [128, WCOLS], f32r, kind="ExternalInput")
    wb_d = nc.dram_tensor("wbpack", [128, BCOLS], bf16, kind="ExternalInput")
    ones_d = nc.dram_tensor("ones_bf", [1, NT + BC], bf16,
                            kind="ExternalInput")
    out_d = nc.dram_tensor("out", [F, NT], f32, kind="ExternalOutput")

    with tile.TileContext(nc) as tc:
        with (
            tc.tile_pool(name="singles", bufs=1) as singles,
            tc.tile_pool(name="work", bufs=4) as work,
        ):
            wp = singles.tile([128, WCOLS], f32r, tag="wp")
            nc.sync.dma_start(wp[:], wp_d[:])
            wb = singles.tile([128, BCOLS], bf16, tag="wb")
            nc.sync.dma_start(wb[:], wb_d[:])

            def wslice(name, rows, g, H):
                if name in BOFF:
                    o = BOFF[name]
                    return wb[0:rows, o + g * H: o + (g + 1) * H]
                o = WOFF[name]
                return wp[0:rows, o + g * H: o + (g + 1) * H]

            # --- state buffers ---
            # big_a serves as h1_seq (encoder) then h4_seq (decoder).
            # Column layout: col (t+1)*32 .. +32 holds h_t; cols 0:32 zero.
            # x_in holds the (augmented) input; h3seq the decoder-L1 output
            # sequence, cols shifted by +BC, with a DMA'd bf16 ones row.
            big_a = singles.tile([H1, NT + BC], bf16, tag="big_a")
            x_in = singles.tile([H2 + 1, NT], f32r, tag="x_in")
            h3seq = singles.tile([H2 + 1, NT + BC], bf16, tag="h3seq")
            h2a = singles.tile([H2 + 1, BC], bf16, tag="h2a")
            z_rep = singles.tile([H2 + 1, BLKC], bf16, tag="z_rep")
            c_big = singles.tile([H1, BC], f32, tag="c_big")
            c_sm = singles.tile([H2, BC], f32, tag="c_sm")
            _CACHE["dbg_tiles"] = {"big_a": big_a, "h3seq": h3seq,
                                   "h2a": h2a, "z_rep": z_rep, "x_in": x_in}

            # First x chunk is a single block so phase A starts as soon as
            # possible; the rest stream in behind it (block-aligned so each
            # input matmul depends on exactly one DMA).
            xblk = [0, 1]
            per = (NBLK - 1 + XCHUNKS - 1) // XCHUNKS
            while xblk[-1] < NBLK:
                xblk.append(min(xblk[-1] + per, NBLK))
            for lo, hi in zip(xblk[:-1], xblk[1:]):
                nc.sync.dma_start(x_in[:, lo * BLKC:hi * BLKC],
                                  xa[:, lo * BLKC:hi * BLKC])
            nc.sync.dma_start(h3seq[H2:H2 + 1, :], ones_d[:])

            def lstm_step(nc, tag, ps, cs, H, wr_g, hprev, c_t, h_out):
                """One recurrence step on psum block ps / col slice cs.

                Plane order (g,i,f,o); g's matmul is emitted last so the
                sigmoid (planes 1:4) can start one matmul earlier. relu(g)*i
                is one DVE STT reading the g plane from PSUM.
                """
                for g in (1, 2, 3, 0):
                    nc.tensor.matmul(
                        ps[:, g, cs], wr_g(g), hprev,
                        start=False, stop=True, skip_group_check=True,
                    )
                act = work.tile([H, 3, BC], f32, tag="act" + tag)
                nc.scalar.activation(act[:], ps[:, 1:4, cs], AF.Sigmoid)
                u = work.tile([H, BC], f32, tag="u" + tag)
                nc.vector.scalar_tensor_tensor(
                    u[:], ps[:, 0, cs], 0.0, act[:, 0, :], ALU.max, ALU.mult)
                # f*c on GPSIMD: off the critical path (only consumed at c+u,
                # after the PSUM-reading STT above), relieves the saturated DVE
                nc.gpsimd.tensor_mul(c_t[:], act[:, 1, :], c_t[:])
                nc.vector.tensor_add(c_t[:], c_t[:], u[:])
                nc.vector.tensor_mul(h_out, act[:, 2, :], c_t[:])

            # PSUM start=True zeroes the whole 2KB zero-region (bank). With
            # BLKC=256 each plane is half a bank, so only the first plane of
            # each bank may start the group; its sibling accumulates onto the
            # freshly zeroed bytes.
            def in_mm(ps, wk_name, wk_rows, H, xr):
                for g in range(4):
                    nc.tensor.matmul(
                        ps[:, g, :], wslice(wk_name, wk_rows, g, H), xr,
                        start=(g % 2 == 0), stop=False, skip_group_check=True,
                    )

            # ================= encoder: A (x->h1) + B (h1->z) wavefront ===
            nc.vector.memset(c_big[:], 0.0)
            nc.vector.memset(c_sm[:], 0.0)
            nc.vector.memset(big_a[:, 0:BC], 0.0)
            nc.vector.memset(h2a[0:H2, :], 0.0)
            nc.vector.memset(h2a[H2:H2 + 1, :], 1.0)

            wr1_g = lambda g: wslice("wr1", 128, g, H1)
            wr2_g = lambda g: wslice("wr2", 65, g, H2)

            # Emission order inside a round is engineered around PE program
            # order: the leading chain's step-0 matmuls go first (they gate
            # the round); the trailing chain's input-matmul burst follows
            # (it depends on the same h as step 0, and runs inside step 0's
            # latency gap); the leading chain's next-block input matmuls are
            # spread one plane per step over s=1..4 (their PSUM slot was
            # freed a full round earlier, so they never block).
            # Trailing chains take their input-side matmuls PER STEP (4 bf16
            # N=32 matmuls, same PE cost as block-batching) so each step
            # depends only on the upstream h one step back - no block-sized
            # boundary dependency, LAG of a single step.
            # Trailing-chain input matmuls: 4-step sub-blocks in a single
            # 1-bank PSUM tile (bufs=3). The [*, 4, 128] tile is one 2KB
            # zero-region, so only plane 0 starts the group.
            SBT = SB // 2

            def in_mm_t(ps, wk_name, wk_rows, H, xr):
                for g in range(4):
                    nc.tensor.matmul(
                        ps[:, g, :], wslice(wk_name, wk_rows, g, H), xr,
                        start=(g == 0), stop=False, skip_group_check=True,
                    )

            LAG = SB
            with tc.tile_pool(name="psum_enc", bufs=2, space="PSUM") as pse:
                psA = {}
                psA[0] = pse.tile([H1, 4, BLKC], f32, tag="psA", name="psA0")
                in_mm(psA[0], "wk1", 65, H1, x_in[:, 0:BLKC])
                psBd = {}
                for k in range(NBLK + 1):
                    if k + 1 < NBLK:
                        psA[k + 1] = pse.tile([H1, 4, BLKC], f32, tag="psA",
                                              name=f"psA{k + 1}")
                    for s in range(SB):
                        cs = slice(s * BC, (s + 1) * BC)
                        if k < NBLK:
                            t = k * SB + s
                            lstm_step(
                                nc, "A", psA[k], cs, H1, wr1_g,
                                big_a[0:H1, t * BC:(t + 1) * BC], c_big,
                                big_a[0:H1, (t + 1) * BC:(t + 2) * BC])
                        if s == 0 and 1 <= k <= NBLK:
                            j = k - 1
                            psBd[j] = pse.tile([H2, 4, BLKC], f32,
                                               tag="psB", name=f"psB{j}")
                            in_mm(psBd[j], "wk2", 128, H2,
                                  big_a[:, j * BLKC + BC:(j + 1) * BLKC + BC])
                        bt = k * SB + s - LAG
                        if 0 <= bt < T:
                            jb, sb = bt // SB, bt % SB
                            lstm_step(
                                nc, "B", psBd[jb],
                                slice(sb * BC, (sb + 1) * BC), H2, wr2_g,
                                h2a[:], c_sm, h2a[0:H2, :])
                            if sb == SB - 1:
                                psBd.pop(jb)
                        if 1 <= s <= 4 and k + 1 < NBLK:
                            g = s - 1
                            nc.tensor.matmul(
                                psA[k + 1][:, g, :], wslice("wk1", 65, g, H1),
                                x_in[:, (k + 1) * BLKC:(k + 2) * BLKC],
                                start=(g % 2 == 0), stop=False,
                                skip_group_check=True)
                    psA.pop(k, None)

            # ============ z_rep + decoder init =============================
            import os
            enc_only = bool(os.environ.get("KERNEL_ENC_ONLY"))
            for s in range(SB):
                nc.vector.tensor_copy(z_rep[:, s * BC:(s + 1) * BC], h2a[:])
            nc.vector.memset(c_sm[:], 0.0)
            nc.vector.memset(c_big[:], 0.0)
            nc.vector.memset(big_a[:, 0:BC], 0.0)
            nc.vector.memset(h3seq[0:H2, 0:BC], 0.0)

            wd1r_g = lambda g: wslice("wd1r", 64, g, H2)
            wd2r_g = lambda g: wslice("wd2r", 128, g, H1)

            # ============= decoder: C (z->h3) + D (h3->h4) wavefront ======
            w_out = wb[0:128, BOFF["wout"]:BOFF["wout"] + F]
            b_out = wp[0:F, WOFF["bout"]:WOFF["bout"] + 1]
            with tc.tile_pool(name="psum_dec", bufs=2, space="PSUM") as psd:
                psCd = {}
                psDd = {}
                for k in range(0 if enc_only else NBLK + 2):
                    for s in range(SB):
                        cs = slice(s * BC, (s + 1) * BC)
                        if k < NBLK:
                            t = k * SB + s
                            jc, sc = t // SBT, t % SBT
                            if sc == 0:
                                psCd[jc] = psd.tile(
                                    [H2, 4, SBT * BC], f32, tag="psC",
                                    name=f"psC{jc}")
                                in_mm_t(psCd[jc], "wd1k", 65, H2,
                                        z_rep[:, 0:SBT * BC])
                            lstm_step(
                                nc, "C", psCd[jc],
                                slice(sc * BC, (sc + 1) * BC), H2, wd1r_g,
                                h3seq[0:H2, t * BC:(t + 1) * BC], c_sm,
                                h3seq[0:H2, (t + 1) * BC:(t + 2) * BC])
                            if sc == SBT - 1:
                                psCd.pop(jc)
                        if s == 0 and 1 <= k <= NBLK:
                            j = k - 1
                            psDd[j] = psd.tile([H1, 4, BLKC], f32,
                                               tag="psD", name=f"psD{j}")
                            in_mm(psDd[j], "wd2k", 65, H1,
                                  h3seq[:, j * BLKC + BC:(j + 1) * BLKC + BC])
                        dt_ = k * SB + s - LAG
                        if 0 <= dt_ < T:
                            jd, sd = dt_ // SB, dt_ % SB
                            lstm_step(
                                nc, "D", psDd[jd],
                                slice(sd * BC, (sd + 1) * BC), H1, wd2r_g,
                                big_a[0:H1, dt_ * BC:(dt_ + 1) * BC], c_big,
                                big_a[0:H1, (dt_ + 1) * BC:(dt_ + 2) * BC])
                            if sd == SB - 1:
                                psDd.pop(jd)
                        if s == 2 and 2 <= k < NBLK + 2:
                            je = k - 2
                            pd = psd.tile([F, BLKC], f32, tag="psE",
                                          name=f"psE{je}")
                            nc.tensor.matmul(
                                pd[:], w_out,
                                big_a[:, je * BLKC + BC:(je + 1) * BLKC + BC],
                                start=True, stop=True,
                            )
                            ob = work.tile([F, BLKC], f32, tag="ob")
                            nc.scalar.activation(ob[:], pd[:], AF.Identity,
                                                 bias=b_out)
                            nc.sync.dma_start(
                                out_d[:, je * BLKC:(je + 1) * BLKC], ob[:])

    _split_excess_waits(nc, mybir)
    return nc


def _split_excess_waits(nc, mybir, limits=None):
    """walrus's PE codegen (S3_LW struct) accepts a single sync-wait per
    matmul; Tile sometimes emits 2+. Move excess waits onto a preceding
    sequencer NoOp on the same engine (executed in order before the
    instruction, so semantics are preserved)."""
    exempt = ()
    for bb in nc.main_func.blocks:
        il = bb.instructions
        pos = 0
        while pos < len(il):
            ins = il[pos]
            limit = None if isinstance(ins, exempt) else 1
            si = ins.sync_info
            if limit is not None and si is not None and len(si.on_wait) > limit:
                keep = list(si.on_wait)[-limit:]
                spill = list(si.on_wait)[:-limit]
                for w in spill:
                    nop = mybir.InstNoOp(
                        name=nc.get_next_instruction_name(),
                        text_hint="wait_split",
                        engine=ins.engine,
                        bass_nofuse=True,
                        sync_info=mybir.SyncInfo(on_wait=[w], on_update=[]),
                    )
                    il.insert(pos, nop)
                    pos += 1
                ins.sync_info = mybir.SyncInfo(
                    on_wait=keep, on_update=list(si.on_update))
            pos += 1


def _get_nc():
    if "nc" not in _CACHE:
        _CACHE["nc"] = _build()
    return _CACHE["nc"]


def _prep_weights(Wk1, Wr1, b1, Wk2, Wr2, b2, Wd1k, Wd1r, bd1, Wd2k, Wd2r,
                  bd2, Wout, bout):
    def perm(W, H):
        Din = W.shape[0]
        return W.reshape(Din, 4, H)[:, PERM, :].reshape(Din, 4 * H)

    def aug(W, b, H):
        return perm(np.concatenate([W, b[None, :]], axis=0), H)

    import ml_dtypes

    mats = {
        "wk1": aug(Wk1, b1, H1),
        "wr1": perm(Wr1, H1),
        "wk2": perm(Wk2, H2),
        "wr2": aug(Wr2, b2, H2),
        "wd1k": aug(Wd1k, bd1, H2),
        "wd1r": perm(Wd1r, H2),
        "wd2k": aug(Wd2k, bd2, H1),
        "wd2r": perm(Wd2r, H1),
        "wout": Wout,
        "bout": np.asarray(bout).reshape(F, 1),
    }
    wpack = np.zeros((128, WCOLS), np.float32)
    for name, rows, cols in WSEGS:
        m = np.asarray(mats[name], np.float32)
        assert m.shape == (rows, cols), (name, m.shape)
        wpack[0:rows, WOFF[name]:WOFF[name] + cols] = m
    wbpack = np.zeros((128, BCOLS), ml_dtypes.bfloat16)
    for name, rows, cols in BSEGS:
        m = np.asarray(mats[name], np.float32)
        assert m.shape == (rows, cols), (name, m.shape)
        wbpack[0:rows, BOFF[name]:BOFF[name] + cols] = m.astype(
            ml_dtypes.bfloat16)
    return wpack, wbpack


def kernel(x, Wk1, Wr1, b1, Wk2, Wr2, b2, Wd1k, Wd1r, bd1, Wd2k, Wd2r, bd2,
           Wout, bout, _run_kwargs=None):
    from concourse.bass_utils import run_bass_kernel_spmd

    import ml_dtypes

    nc = _get_nc()
    wpack, wbpack = _prep_weights(
        np.asarray(Wk1), np.asarray(Wr1), np.asarray(b1),
        np.asarray(Wk2), np.asarray(Wr2), np.asarray(b2),
        np.asarray(Wd1k), np.asarray(Wd1r), np.asarray(bd1),
        np.asarray(Wd2k), np.asarray(Wd2r), np.asarray(bd2),
        np.asarray(Wout), np.asarray(bout))
    ones_bf = np.ones((1, NT + BC), ml_dtypes.bfloat16)

    x = np.asarray(x, dtype=np.float32)
    in_maps = []
    for i in range(NCORES):
        xs = x[i * BC:(i + 1) * BC]                 # [32, 512, 64]
        xt = xs.transpose(2, 1, 0).reshape(F, NT)   # [64, (t,b)]
        xaug = np.concatenate([xt, np.ones((1, NT), np.float32)], axis=0)
        in_maps.append({"xa": np.ascontiguousarray(xaug), "wpack": wpack,
                        "wbpack": wbpack, "ones_bf": ones_bf})

    kwargs = _run_kwargs or {}
    res = run_bass_kernel_spmd(nc, in_maps, list(range(NCORES)), **kwargs)
    _CACHE["last_results"] = res

    out = np.empty((B, T, F), np.float32)
    for i in range(NCORES):
        o = np.asarray(res.results[i]["out"]).reshape(F, T, BC)
        out[i * BC:(i + 1) * BC] = o.transpose(2, 1, 0)
    return out
